# revision 39
# baseline (speedup 1.0000x reference)
"""Blockwise-parallel transformer layer on 8 TRN2 NeuronCores.

Sharding: by kv-head (the reference's einsum ties kv-head to seq pos mod 16).
Core c owns heads {2c, 2c+1} and the 256 seq rows n with n%16 in {2c, 2c+1}.
K/V projections therefore only need the 128-wide Wk/Wv column slice for the
core's two heads (8x less replicated GEMM work than seq-sharding, and no
collectives).  Scores run twice: q-major for the per-block max (DVE
reduce_max), kc-major for exp/num so no e-transposes are needed; the block
max is broadcast into the kc-major psum with an indicator matmul, and den
falls out of a ones-column augmented into V.

Shapes (hardcoded): x (1, 2048, 1024); Wq/Wk/Wv (1024, 1024); W1 (4096,
1024); W2 (1024, 4096); H=16 heads * HD=64; KB=16 kv blocks of 128.
"""

import sys
from contextlib import ExitStack

import numpy as np

for _p in ("/opt/trn_rl_repo", "/root/.axon_site/_ro/trn_rl_repo"):
    if _p not in sys.path:
        sys.path.append(_p)

import concourse.bass as bass  # noqa: E402
import concourse.tile as tile  # noqa: E402
from concourse import bacc, mybir  # noqa: E402
from concourse._compat import with_exitstack  # noqa: E402
from concourse.bass import ds  # noqa: E402
from concourse.bass_utils import run_bass_kernel_spmd  # noqa: E402
from concourse.masks import make_identity  # noqa: E402

D = 1024
H = 16
HD = 64
FF = 4096
N = 2048
KB = 16
NCORES = 8
RQ = N // NCORES  # 256 local rows
P = 128

F32 = mybir.dt.float32
F32R = mybir.dt.float32r
BF16 = mybir.dt.bfloat16
AX = mybir.AxisListType
AF = mybir.ActivationFunctionType


@with_exitstack
def _tile_kernel(ctx: ExitStack, tc: tile.TileContext, io: dict):
    nc = tc.nc

    consts = ctx.enter_context(tc.tile_pool(name="consts", bufs=1))
    ident = consts.tile([P, P], F32)
    make_identity(nc, ident)
    identr = consts.tile([P, P], F32R)
    nc.scalar.activation(out=identr, in_=ident, func=AF.Copy)
    bqs = consts.tile([HD, 16], F32)
    bks = consts.tile([HD, 2], F32)
    bvs = consts.tile([HD, 2], F32)
    b1s = consts.tile([P, 32], F32)
    b2s = consts.tile([P, 8], F32)
    nc.sync.dma_start(out=bqs, in_=io["bq2d"])
    nc.sync.dma_start(out=bks, in_=io["bk2d"])
    nc.sync.dma_start(out=bvs, in_=io["bv2d"])
    nc.sync.dma_start(out=b1s, in_=io["b12d"])
    nc.sync.dma_start(out=b2s, in_=io["b22d"])

    persist = ctx.enter_context(tc.tile_pool(name="persist", bufs=1))
    attn = persist.tile([P, 2, D], F32)  # [Q, h, (g,f)]
    xl = persist.tile([P, 2, D], F32)  # local x rows (residual), [Q, h, d]
    h1 = persist.tile([P, 2, D], F32)
    h1T = persist.tile([P, 8, RQ], F32)
    h1Tb = persist.tile([P, 8, RQ], BF16)  # bf16 copy for GEMM1 rhs
    nc.sync.dma_start(out=xl[:, 0, :], in_=io["xloc"][0:P, :])
    nc.sync.dma_start(out=xl[:, 1, :], in_=io["xloc"][P : 2 * P, :])

    with tc.tile_pool(name="kvq", bufs=1) as kvp:
        # kTind rows 0-63: k for head h; rows 64-79: block indicator.
        # qTnm rows 0-63: q channels; rows 64-79: -blockmax rows (pass A).
        kTind = kvp.tile([HD + KB, 2, N], F32R)
        vaug = kvp.tile([P, KB, 2, HD + 1], F32R)
        qTnm = kvp.tile([HD + KB, 2, 16, P], F32R)
        nm = kvp.tile([P, 2, 16, KB], F32)  # -max per [Q, h, g, K]
        # IND[j, kc] = 1 iff j == kc//128 — block-max broadcast rows 64-79
        with tc.tile_pool(name="tmp_ind", bufs=1) as tmpp:
            indf = tmpp.tile([KB, KB * P], F32)
            nc.gpsimd.memset(indf, 1.0)
            nc.gpsimd.affine_select(
                out=indf, in_=indf, compare_op=mybir.AluOpType.is_ge, fill=0.0,
                base=0, pattern=[[1, KB * P]], channel_multiplier=-P,
            )
            nc.gpsimd.affine_select(
                out=indf, in_=indf, compare_op=mybir.AluOpType.is_ge, fill=0.0,
                base=P - 1, pattern=[[-1, KB * P]], channel_multiplier=P,
            )
            for h in range(2):
                nc.scalar.activation(
                    out=kTind[HD : HD + KB, h, :], in_=indf, func=AF.Copy
                )
        with tc.tile_pool(name="wstream", bufs=2) as wsp:
            h1p = (ident, xl, h1, h1T, h1Tb)
            wtiles = []
            _attention(
                tc, io, kvp, identr, bqs, bks, bvs, kTind, vaug, qTnm, nm,
                attn, wsp, h1p, wtiles,
            )
            _ffn_phase(tc, io, b1s, b2s, h1T, h1Tb, wsp, wtiles)


def _kvq_proj(tc, io, kvp, identr, bks, bvs, kT2, vaug):
    # kT2 is the full kTind tile; only rows 0-63 are written here.
    nc = tc.nc
    NCH = 4
    CW = N // NCH  # 512
    ones32 = kvp.tile([P, KB * 2], F32)
    nc.gpsimd.memset(ones32, 1.0)
    # den ones column at f=64 of every (K, h) slot
    nc.scalar.activation(
        out=vaug[:, :, :, HD : HD + 1].rearrange("p a b c -> p (a b c)"),
        in_=ones32,
        func=AF.Copy,
    )

    # K/V projections: only this core's 2-head dout slice (128 cols)
    with (
        tc.tile_pool(name="vt2", bufs=1) as vt2p,
        tc.tile_pool(name="wkv", bufs=1) as wp,
        tc.tile_pool(name="xs", bufs=2) as xsp,
        tc.psum_pool(name="ps_kv", bufs=2) as pskv,
    ):
        vT2 = vt2p.tile([HD, 2, N], F32R)
        wk = wp.tile([P, 8, P], F32R)
        wv = wp.tile([P, 8, P], F32R)
        nc.sync.dma_start(out=wk, in_=io["wkT"].rearrange("(j p) c -> p j c", p=P))
        nc.sync.dma_start(out=wv, in_=io["wvT"].rearrange("(j p) c -> p j c", p=P))
        xTr = io["xT"].rearrange("(j p) n -> p j n", p=P)
        for c in range(NCH):
            xc = xsp.tile([P, 8, CW], F32R, tag="xc")
            nc.sync.dma_start(out=xc, in_=xTr[:, :, ds(c * CW, CW)])
            for w, bias_t, dst in ((wk, bks, kT2), (wv, bvs, vT2)):
                ps = pskv.tile([P, CW], F32, tag="pkv")
                for dx in range(8):
                    nc.tensor.matmul(
                        ps,
                        lhsT=w[:, dx, :],
                        rhs=xc[:, dx, :],
                        start=(dx == 0),
                        stop=(dx == 7),
                    )
                for h in range(2):
                    nc.vector.tensor_scalar_add(
                        out=dst[0:HD, h, ds(c * CW, CW)],
                        in0=ps[ds(h * HD, HD), :],
                        scalar1=bias_t[:, h : h + 1],
                    )

        # vT2 -> vaug (n-major) via PE transposes
        with tc.psum_pool(name="ps_vt", bufs=2) as psvt:
            for h in range(2):
                for K in range(KB):
                    vt = psvt.tile([P, HD], F32R, tag="vt")
                    nc.tensor.transpose(
                        vt, vT2[:, h, ds(K * P, P)], identr[0:HD, 0:HD]
                    )
                    nc.vector.tensor_copy(out=vaug[:, K, h, 0:HD], in_=vt)

def _attention(
    tc, io, kvp, identr, bqs, bks, bvs, kTind, vaug, qTnm, nm, attn, wsp,
    h1p, wtiles,
):
    """Pass A (q-major scores -> per-block -max, DVE-bound) software-pipelined
    under pass B (kc-major scores+max-broadcast in ONE matmul via the
    indicator rows stacked at partitions 64-79, then exp -> num/den matmul).

    Emission order interleaves A-units (one g at a time) ahead of the B
    chunks that consume their -max rows, so DVE reductions hide under PE.
    The Q projection is fused in front, with the first 8 A-units interleaved
    so DVE starts early; h1/h1T for each head are built as soon as its
    attention output finalizes.
    """
    nc = tc.nc
    ident, xl, h1, h1T, h1Tb = h1p
    nsb = kvp.tile([HD + 1, 2, N], F32)  # [f(+den), h, (g,Q)]

    _kvq_proj(tc, io, kvp, identr, bks, bvs, kTind, vaug)

    with (
        tc.tile_pool(name="et", bufs=3) as etp,
        tc.tile_pool(name="fin", bufs=3) as finp,
        tc.psum_pool(name="ps_a", bufs=2) as psa,
        tc.psum_pool(name="ps_t", bufs=1) as pst,
    ):

        def a_mm(h, g):
            # q-major scores for (h, g); -max per kv block (DVE)
            for sh in range(4):
                s = psa.tile([P, 4 * P], F32, tag="s")
                nc.tensor.matmul(
                    s,
                    lhsT=qTnm[0:HD, h, g, :],
                    rhs=kTind[0:HD, h, ds(sh * 512, 512)],
                    start=True,
                    stop=True,
                )
                nc.vector.reduce_max(
                    out=nm[:, h, g, ds(sh * 4, 4)],
                    in_=s.rearrange("p (b f) -> p b f", f=P),
                    axis=AX.X,
                    negate=True,
                )

        def a_nt(h, g):
            # -max rows into qTnm partitions 64-79 (deferred so PE does not
            # stall in-queue behind the DVE reduces)
            nt = pst.tile([P, P], F32, tag="t")
            nc.tensor.transpose(nt[0:KB, :], nm[:, h, g, :], ident)
            nc.scalar.activation(
                out=qTnm[HD : HD + KB, h, g, :], in_=nt[0:KB, :], func=AF.Copy
            )

        nonlocal_pools = {}

        def b_chunk(h, qc):
            # 512 q-things (4 g), all kv blocks in pairs; one exp per pair;
            # st pairs emitted ahead of num so PE never waits on ACT.
            psst = nonlocal_pools["psst"]
            psn = nonlocal_pools["psn"]
            nacc = psn.tile([HD + 1, 512], F32, tag="nacc")
            ets = []

            def mm_st(p):
                st = psst.tile([P, 2, 512], F32, tag="st")
                for i in range(2):
                    nc.tensor.matmul(
                        st[:, i, :],
                        lhsT=kTind[:, h, ds((2 * p + i) * P, P)],
                        rhs=qTnm[:, h, ds(qc * 4, 4), :],
                        start=True,
                        stop=True,
                    )
                et = etp.tile([P, 2, 512], F32R, tag="et")
                nc.scalar.activation(out=et, in_=st, func=AF.Exp)
                ets.append(et)

            def mm_num(p):
                for i in range(2):
                    K = 2 * p + i
                    nc.tensor.matmul(
                        nacc,
                        lhsT=vaug[:, K, h, :],
                        rhs=ets[p][:, i, :],
                        start=(K == 0),
                        stop=(K == KB - 1),
                    )

            NP2 = KB // 2
            mm_st(0)
            mm_st(1)
            for p in range(NP2):
                if p + 2 < NP2:
                    mm_st(p + 2)
                mm_num(p)
            nc.scalar.activation(
                out=nsb[:, h, ds(qc * 512, 512)], in_=nacc, func=AF.Identity
            )

        def fin_unit(h, g):
            # transpose num/den back to Q-partitions, scale by 1/den
            tr = pst.tile([P, P], F32, tag="t")
            nc.tensor.transpose(
                tr[:, 0 : HD + 1], nsb[:, h, ds(g * P, P)],
                ident[0 : HD + 1, 0 : HD + 1],
            )
            rcp = finp.tile([P, 1], F32, tag="rcp")
            nc.vector.reciprocal(out=rcp, in_=tr[:, HD : HD + 1])
            nc.scalar.activation(
                out=attn[:, h, ds(g * HD, HD)], in_=tr[:, 0:HD],
                func=AF.Identity, scale=rcp,
            )

        def h1_unit(h):
            # h1 = attn + x for head h, transposed into h1T/h1Tb columns
            nc.vector.tensor_add(
                out=h1[:, h, :], in0=attn[:, h, :], in1=xl[:, h, :]
            )
            for dc in range(8):
                tps = pst.tile([P, P], F32, tag="t")
                nc.tensor.transpose(tps, h1[:, h, ds(dc * P, P)], ident)
                nc.vector.tensor_copy(out=h1T[:, dc, ds(h * P, P)], in_=tps)
                nc.scalar.activation(
                    out=h1Tb[:, dc, ds(h * P, P)], in_=tps, func=AF.Copy
                )

        # Q projection (g-pairs; dout 128 at a time), pre-scaled 1/8
        # host-side; the first 8 A-units interleave so DVE starts early.
        with (
            tc.tile_pool(name="wq", bufs=2) as wqp,
            tc.tile_pool(name="xtl", bufs=1) as xtlp,
            tc.psum_pool(name="ps_q", bufs=2) as psq,
        ):
            xtl = xtlp.tile([P, 8, RQ], F32R)
            nc.sync.dma_start(
                out=xtl, in_=io["xTloc"].rearrange("(j p) n -> p j n", p=P)
            )
            wqr = io["wqT"].rearrange("(j p) c -> p j c", p=P)
            for half in range(2):
                wq = wqp.tile([P, 8, 4 * P], F32R, tag="wq")
                nc.sync.dma_start(
                    out=wq, in_=wqr[:, :, ds(half * 4 * P, 4 * P)]
                )
                for t in range(4):
                    gp = half * 4 + t  # g-pair index; g = 2*gp, 2*gp+1
                    ps = psq.tile([P, RQ], F32, tag="pq")
                    for dx in range(8):
                        nc.tensor.matmul(
                            ps,
                            lhsT=wq[:, dx, ds(t * P, P)],
                            rhs=xtl[:, dx, :],
                            start=(dx == 0),
                            stop=(dx == 7),
                        )
                    for gh in range(2):
                        g = 2 * gp + gh
                        nc.scalar.activation(
                            out=qTnm[0:HD, :, g, :],
                            in_=ps[ds(gh * HD, HD), :].rearrange(
                                "p (h q) -> p h q", h=2
                            ),
                            func=AF.Identity,
                            bias=bqs[:, g : g + 1],
                        )
                    if half == 0:
                        a_mm(0, 2 * gp)
                        a_mm(0, 2 * gp + 1)

        # FFN weight chunks 0-1 stream during attention
        wtiles.append(_ffn_weight_dma(tc, io, wsp, 0))
        wtiles.append(_ffn_weight_dma(tc, io, wsp, 1))

        # chunk c consumes g-set G(c); a_mm runs 2 chunks ahead (G0/G1 were
        # emitted in the Q loop), a_nt 1 ahead, fin one behind; h1 for a head
        # follows its last fin batch.
        chunks = [(h, qc) for h in range(2) for qc in range(4)]
        gsets = [[(h, qc * 4 + j) for j in range(4)] for h, qc in chunks]
        with (
            tc.psum_pool(name="ps_st", bufs=2) as psst_,
            tc.psum_pool(name="ps_n", bufs=1) as psn_,
        ):
            nonlocal_pools["psst"] = psst_
            nonlocal_pools["psn"] = psn_
            for hg in gsets[0]:
                a_nt(*hg)
            for c, (h, qc) in enumerate(chunks):
                if c + 2 < len(chunks):
                    for hg in gsets[c + 2]:
                        a_mm(*hg)
                if c + 1 < len(chunks):
                    for hg in gsets[c + 1]:
                        a_nt(*hg)
                b_chunk(h, qc)
                if c > 0:
                    for hg in gsets[c - 1]:
                        fin_unit(*hg)
                    if gsets[c - 1][-1][1] == 15:  # last g of a head done
                        h1_unit(gsets[c - 1][0][0])
            for hg in gsets[-1]:
                fin_unit(*hg)
            h1_unit(1)


def _ffn_weight_dma(tc, io, wsp, chunk):
    # one batched DMA per 1024-col bf16 weight chunk; chunks 0-3 = W1
    # quarters, 4-7 = W2 quarters
    nc = tc.nc
    w = wsp.tile([P, 8, 8 * P], BF16, tag="wbig")
    if chunk < 4:
        src = io["w1T"].rearrange("(j p) c -> p j c", p=P)[
            :, :, ds(chunk * 8 * P, 8 * P)
        ]
    else:
        q2 = chunk - 4
        src = io["w2T"][ds(q2 * 8 * P, 8 * P), :].rearrange(
            "(j p) c -> p j c", p=P
        )
    nc.sync.dma_start(out=w, in_=src)
    return w


def _ffn_phase(tc, io, b1s, b2s, h1T, h1Tb, wsp, wtiles):
    nc = tc.nc
    with (
        tc.tile_pool(name="ffn", bufs=1) as fp,
        tc.tile_pool(name="ffn_sm", bufs=3) as fsm,
    ):
        hid = fp.tile([P, 32, RQ], BF16)
        with tc.psum_pool(name="ps_f", bufs=3) as psf:
            for q4 in range(4):  # W1 column quarters [128, 8, 1024]
                wtiles.append(_ffn_weight_dma(tc, io, wsp, q4 + 2))
                w1q = wtiles[q4]
                for f in range(8):
                    ff = q4 * 8 + f
                    ps = psf.tile([P, RQ], F32, tag="fps")
                    for dc in range(8):
                        nc.tensor.matmul(
                            ps,
                            lhsT=w1q[:, dc, ds(f * P, P)],
                            rhs=h1Tb[:, dc, :],
                            start=(dc == 0),
                            stop=(dc == 7),
                        )
                    nc.scalar.activation(
                        out=hid[:, ff, :], in_=ps, func=AF.Relu,
                        bias=b1s[:, ff : ff + 1],
                    )
        with tc.psum_pool(name="ps_y", bufs=1) as psy:
            yaccs = [
                psy.tile([P, RQ], F32, tag=f"y{dy}", name=f"yacc{dy}")
                for dy in range(8)
            ]
            for q2 in range(4):  # W2 ffc-quarters [128, 8, 1024]
                if q2 + 6 < 8:
                    wtiles.append(_ffn_weight_dma(tc, io, wsp, q2 + 6))
                w2q = wtiles[4 + q2]
                for dy in range(8):
                    for fc in range(8):
                        nc.tensor.matmul(
                            yaccs[dy],
                            lhsT=w2q[:, fc, ds(dy * P, P)],
                            rhs=hid[:, q2 * 8 + fc, :],
                            start=(q2 == 0 and fc == 0),
                            stop=(q2 == 3 and fc == 7),
                        )
            for dy in range(8):
                ysb = fsm.tile([P, RQ], F32, tag="ysb")
                nc.scalar.activation(
                    out=ysb, in_=yaccs[dy], func=AF.Identity,
                    bias=b2s[:, dy : dy + 1],
                )
                osb = fsm.tile([P, RQ], F32, tag="osb")
                nc.vector.tensor_add(out=osb, in0=ysb, in1=h1T[:, dy, :])
                nc.sync.dma_start(out=io["outT"][ds(dy * P, P), :], in_=osb)


def _build():
    nc = bacc.Bacc(
        "TRN2", target_bir_lowering=False, debug=False, num_devices=NCORES
    )
    io = {}
    def inp(name, shape, dt=F32):
        io[name] = nc.dram_tensor(name, shape, dt, kind="ExternalInput").ap()
    inp("xT", [D, N], F32R)
    inp("xTloc", [D, RQ], F32R)
    inp("xloc", [RQ, D])
    inp("wqT", [D, D], F32R)
    inp("wkT", [D, P], F32R)
    inp("wvT", [D, P], F32R)
    inp("w1T", [D, FF], BF16)
    inp("w2T", [FF, D], BF16)
    inp("bq2d", [HD, 16])
    inp("bk2d", [HD, 2])
    inp("bv2d", [HD, 2])
    inp("b12d", [P, 32])
    inp("b22d", [P, 8])
    io["outT"] = nc.dram_tensor("outT", [D, RQ], F32, kind="ExternalOutput").ap()
    with tile.TileContext(nc) as tc:
        _tile_kernel(tc, io)
    nc.compile()
    return nc


_CACHE = {}


def _get_nc():
    if "nc" not in _CACHE:
        _CACHE["nc"] = _build()
    return _CACHE["nc"]


_BF16NP = mybir.dt.np(BF16)


def _rows_for_core(c):
    # local row r = h*128 + Q  ->  global n = Q*16 + 2c + h
    r = np.arange(RQ)
    h, Q = r // P, r % P
    return Q * 16 + 2 * c + h


def make_in_maps(inputs):
    x = np.ascontiguousarray(np.asarray(inputs["x"], np.float32)[0])
    xT = np.ascontiguousarray(x.T)

    wqT8 = np.ascontiguousarray((np.asarray(inputs["Wq"], np.float32) / 8.0).T)
    bq8 = np.asarray(inputs["bq"], np.float32) / 8.0
    wkT = np.asarray(inputs["Wk"], np.float32).T  # [din, dout]
    wvT = np.asarray(inputs["Wv"], np.float32).T
    bk = np.asarray(inputs["bk"], np.float32)
    bv = np.asarray(inputs["bv"], np.float32)

    def b2d(b, k):
        return np.ascontiguousarray(np.asarray(b, np.float32).reshape(k, P).T)

    common = {
        "xT": xT,
        "wqT": wqT8,
        "bq2d": np.ascontiguousarray(bq8.reshape(16, HD).T),  # [f, g]
        "w1T": np.ascontiguousarray(
            np.asarray(inputs["W1"], np.float32).T.astype(_BF16NP)
        ),
        "w2T": np.ascontiguousarray(
            np.asarray(inputs["W2"], np.float32).T.astype(_BF16NP)
        ),
        "b12d": b2d(inputs["b1"], 32),
        "b22d": b2d(inputs["b2"], 8),
    }
    in_maps = []
    for c in range(NCORES):
        rows = _rows_for_core(c)
        sl = slice(c * P, (c + 1) * P)
        m = dict(common)
        m["xTloc"] = np.ascontiguousarray(xT[:, rows])
        m["xloc"] = np.ascontiguousarray(x[rows])
        m["wkT"] = np.ascontiguousarray(wkT[:, sl])
        m["wvT"] = np.ascontiguousarray(wvT[:, sl])
        m["bk2d"] = np.ascontiguousarray(bk[sl].reshape(2, HD).T)
        m["bv2d"] = np.ascontiguousarray(bv[sl].reshape(2, HD).T)
        in_maps.append(m)
    return in_maps


def kernel(**inputs):
    nc = _get_nc()
    res = run_bass_kernel_spmd(nc, make_in_maps(inputs), core_ids=list(range(NCORES)))
    out = np.empty((1, N, D), np.float32)
    for c in range(NCORES):
        out[0, _rows_for_core(c), :] = res.results[c]["outT"].T
    return out


# revision 48
# speedup vs baseline: 1.0125x; 1.0125x over previous
"""Blockwise-parallel transformer layer on 8 TRN2 NeuronCores.

Sharding: by kv-head (the reference's einsum ties kv-head to seq pos mod 16).
Core c owns heads {2c, 2c+1} and the 256 seq rows n with n%16 in {2c, 2c+1}.
K/V projections therefore only need the 128-wide Wk/Wv column slice for the
core's two heads (8x less replicated GEMM work than seq-sharding, and no
collectives).  Scores run twice: q-major for the per-block max (DVE
reduce_max), kc-major for exp/num so no e-transposes are needed; the block
max is broadcast into the kc-major psum with an indicator matmul, and den
falls out of a ones-column augmented into V.

Shapes (hardcoded): x (1, 2048, 1024); Wq/Wk/Wv (1024, 1024); W1 (4096,
1024); W2 (1024, 4096); H=16 heads * HD=64; KB=16 kv blocks of 128.
"""

import sys
from contextlib import ExitStack

import numpy as np

for _p in ("/opt/trn_rl_repo", "/root/.axon_site/_ro/trn_rl_repo"):
    if _p not in sys.path:
        sys.path.append(_p)

import concourse.bass as bass  # noqa: E402
import concourse.tile as tile  # noqa: E402
from concourse import bacc, mybir  # noqa: E402
from concourse._compat import with_exitstack  # noqa: E402
from concourse.bass import ds  # noqa: E402
from concourse.bass_utils import run_bass_kernel_spmd  # noqa: E402
from concourse.masks import make_identity  # noqa: E402

D = 1024
H = 16
HD = 64
FF = 4096
N = 2048
KB = 16
NCORES = 8
RQ = N // NCORES  # 256 local rows
P = 128

F32 = mybir.dt.float32
F32R = mybir.dt.float32r
BF16 = mybir.dt.bfloat16
AX = mybir.AxisListType
AF = mybir.ActivationFunctionType


@with_exitstack
def _tile_kernel(ctx: ExitStack, tc: tile.TileContext, io: dict):
    nc = tc.nc

    consts = ctx.enter_context(tc.tile_pool(name="consts", bufs=1))
    ident = consts.tile([P, P], F32)
    make_identity(nc, ident)
    identr = consts.tile([P, P], F32R)
    nc.scalar.activation(out=identr, in_=ident, func=AF.Copy)
    bqs = consts.tile([HD, 16], F32)
    bks = consts.tile([HD, 2], F32)
    bvs = consts.tile([HD, 2], F32)
    b1s = consts.tile([P, 32], F32)
    b2s = consts.tile([P, 8], F32)
    nc.sync.dma_start(out=bqs, in_=io["bq2d"])
    nc.sync.dma_start(out=bks, in_=io["bk2d"])
    nc.sync.dma_start(out=bvs, in_=io["bv2d"])
    nc.sync.dma_start(out=b1s, in_=io["b12d"])
    nc.sync.dma_start(out=b2s, in_=io["b22d"])

    persist = ctx.enter_context(tc.tile_pool(name="persist", bufs=1))
    attn = persist.tile([P, 2, D], F32)  # [Q, h, (g,f)]
    xl = persist.tile([P, 2, D], F32)  # local x rows (residual), [Q, h, d]
    h1 = persist.tile([P, 2, D], F32)
    h1T = persist.tile([P, 8, RQ], F32)
    h1Tb = persist.tile([P, 8, RQ], BF16)  # bf16 copy for GEMM1 rhs
    nc.sync.dma_start(out=xl[:, 0, :], in_=io["xloc"][0:P, :])
    nc.sync.dma_start(out=xl[:, 1, :], in_=io["xloc"][P : 2 * P, :])

    with tc.tile_pool(name="kvq", bufs=1) as kvp:
        # kTind rows 0-63: k for head h; rows 64-79: block indicator.
        # qTnm rows 0-63: q channels; rows 64-79: -blockmax rows (pass A).
        kTind = kvp.tile([HD + KB, 2, N], F32R)
        vaug = kvp.tile([P, KB, 2, HD + 1], F32R)
        qTnm = kvp.tile([HD + KB, 2, 16, P], F32R)
        nm = kvp.tile([P, 2, 16, KB], F32)  # -max per [Q, h, g, K]
        # IND[j, kc] = 1 iff j == kc//128 — block-max broadcast rows 64-79
        with tc.tile_pool(name="tmp_ind", bufs=1) as tmpp:
            indf = tmpp.tile([KB, KB * P], F32)
            nc.gpsimd.memset(indf, 1.0)
            nc.gpsimd.affine_select(
                out=indf, in_=indf, compare_op=mybir.AluOpType.is_ge, fill=0.0,
                base=0, pattern=[[1, KB * P]], channel_multiplier=-P,
            )
            nc.gpsimd.affine_select(
                out=indf, in_=indf, compare_op=mybir.AluOpType.is_ge, fill=0.0,
                base=P - 1, pattern=[[-1, KB * P]], channel_multiplier=P,
            )
            for h in range(2):
                nc.scalar.activation(
                    out=kTind[HD : HD + KB, h, :], in_=indf, func=AF.Copy
                )
        with tc.tile_pool(name="wstream", bufs=2) as wsp:
            h1p = (ident, xl, h1, h1T, h1Tb)
            wtiles = []
            _attention(
                tc, io, kvp, identr, bqs, bks, bvs, kTind, vaug, qTnm, nm,
                attn, wsp, h1p, wtiles,
            )
            _ffn_phase(tc, io, b1s, b2s, h1T, h1Tb, wsp, wtiles)


def _kvq_proj(tc, io, kvp, identr, bks, bvs, kT2, vaug):
    # kT2 is the full kTind tile; only rows 0-63 are written here.
    nc = tc.nc
    NCH = 4
    CW = N // NCH  # 512
    ones32 = kvp.tile([P, KB * 2], F32)
    nc.gpsimd.memset(ones32, 1.0)
    # den ones column at f=64 of every (K, h) slot
    nc.scalar.activation(
        out=vaug[:, :, :, HD : HD + 1].rearrange("p a b c -> p (a b c)"),
        in_=ones32,
        func=AF.Copy,
    )

    # K/V projections: only this core's 2-head dout slice (128 cols)
    with (
        tc.tile_pool(name="vt2", bufs=1) as vt2p,
        tc.tile_pool(name="wkv", bufs=1) as wp,
        tc.tile_pool(name="xs", bufs=2) as xsp,
        tc.psum_pool(name="ps_kv", bufs=2) as pskv,
    ):
        vT2 = vt2p.tile([HD, 2, N], F32R)
        wk = wp.tile([P, 8, P], BF16)
        wv = wp.tile([P, 8, P], BF16)
        nc.sync.dma_start(out=wk, in_=io["wkT"].rearrange("(j p) c -> p j c", p=P))
        nc.sync.dma_start(out=wv, in_=io["wvT"].rearrange("(j p) c -> p j c", p=P))
        xTr = io["xT"].rearrange("(j p) n -> p j n", p=P)
        for c in range(NCH):
            xc = xsp.tile([P, 8, CW], BF16, tag="xc")
            nc.sync.dma_start(out=xc, in_=xTr[:, :, ds(c * CW, CW)])
            for w, bias_t, dst in ((wk, bks, kT2), (wv, bvs, vT2)):
                ps = pskv.tile([P, CW], F32, tag="pkv")
                for dx in range(8):
                    nc.tensor.matmul(
                        ps,
                        lhsT=w[:, dx, :],
                        rhs=xc[:, dx, :],
                        start=(dx == 0),
                        stop=(dx == 7),
                    )
                for h in range(2):
                    nc.vector.tensor_scalar_add(
                        out=dst[0:HD, h, ds(c * CW, CW)],
                        in0=ps[ds(h * HD, HD), :],
                        scalar1=bias_t[:, h : h + 1],
                    )

        # vT2 -> vaug (n-major) via PE transposes
        with tc.psum_pool(name="ps_vt", bufs=2) as psvt:
            for h in range(2):
                for K in range(KB):
                    vt = psvt.tile([P, HD], F32R, tag="vt")
                    nc.tensor.transpose(
                        vt, vT2[:, h, ds(K * P, P)], identr[0:HD, 0:HD]
                    )
                    nc.vector.tensor_copy(out=vaug[:, K, h, 0:HD], in_=vt)

def _attention(
    tc, io, kvp, identr, bqs, bks, bvs, kTind, vaug, qTnm, nm, attn, wsp,
    h1p, wtiles,
):
    """Pass A (q-major scores -> per-block -max, DVE-bound) software-pipelined
    under pass B (kc-major scores+max-broadcast in ONE matmul via the
    indicator rows stacked at partitions 64-79, then exp -> num/den matmul).

    Emission order interleaves A-units (one g at a time) ahead of the B
    chunks that consume their -max rows, so DVE reductions hide under PE.
    The Q projection is fused in front, with the first 8 A-units interleaved
    so DVE starts early; h1/h1T for each head are built as soon as its
    attention output finalizes.
    """
    nc = tc.nc
    ident, xl, h1, h1T, h1Tb = h1p
    nsb = kvp.tile([HD + 1, 2, N], F32)  # [f(+den), h, (g,Q)]

    _kvq_proj(tc, io, kvp, identr, bks, bvs, kTind, vaug)

    with (
        tc.tile_pool(name="et", bufs=3) as etp,
        tc.tile_pool(name="fin", bufs=3) as finp,
        tc.psum_pool(name="ps_a", bufs=2) as psa,
        tc.psum_pool(name="ps_t", bufs=1) as pst,
    ):

        def a_mm(h, g):
            # q-major scores for (h, g); -max per kv block (DVE)
            for sh in range(4):
                s = psa.tile([P, 4 * P], F32, tag="s")
                nc.tensor.matmul(
                    s,
                    lhsT=qTnm[0:HD, h, g, :],
                    rhs=kTind[0:HD, h, ds(sh * 512, 512)],
                    start=True,
                    stop=True,
                )
                nc.vector.reduce_max(
                    out=nm[:, h, g, ds(sh * 4, 4)],
                    in_=s.rearrange("p (b f) -> p b f", f=P),
                    axis=AX.X,
                    negate=True,
                )

        def a_nt(h, g):
            # -max rows into qTnm partitions 64-79 (deferred so PE does not
            # stall in-queue behind the DVE reduces)
            nt = pst.tile([P, P], F32, tag="t")
            nc.tensor.transpose(nt[0:KB, :], nm[:, h, g, :], ident)
            nc.scalar.activation(
                out=qTnm[HD : HD + KB, h, g, :], in_=nt[0:KB, :], func=AF.Copy
            )

        nonlocal_pools = {}

        def b_chunk(h, qc):
            # 512 q-things (4 g), all kv blocks in pairs; one exp per pair;
            # st pairs emitted ahead of num so PE never waits on ACT.
            psst = nonlocal_pools["psst"]
            psn = nonlocal_pools["psn"]
            nacc = psn.tile([HD + 1, 512], F32, tag="nacc")
            ets = []

            def mm_st(p):
                st = psst.tile([P, 2, 512], F32, tag="st")
                for i in range(2):
                    nc.tensor.matmul(
                        st[:, i, :],
                        lhsT=kTind[:, h, ds((2 * p + i) * P, P)],
                        rhs=qTnm[:, h, ds(qc * 4, 4), :],
                        start=True,
                        stop=True,
                    )
                et = etp.tile([P, 2, 512], F32R, tag="et")
                nc.scalar.activation(out=et, in_=st, func=AF.Exp)
                ets.append(et)

            def mm_num(p):
                for i in range(2):
                    K = 2 * p + i
                    nc.tensor.matmul(
                        nacc,
                        lhsT=vaug[:, K, h, :],
                        rhs=ets[p][:, i, :],
                        start=(K == 0),
                        stop=(K == KB - 1),
                    )

            NP2 = KB // 2
            mm_st(0)
            mm_st(1)
            for p in range(NP2):
                if p + 2 < NP2:
                    mm_st(p + 2)
                mm_num(p)
            nc.scalar.activation(
                out=nsb[:, h, ds(qc * 512, 512)], in_=nacc, func=AF.Identity
            )

        def fin_unit(h, g):
            # transpose num/den back to Q-partitions, scale by 1/den
            tr = pst.tile([P, P], F32, tag="t")
            nc.tensor.transpose(
                tr[:, 0 : HD + 1], nsb[:, h, ds(g * P, P)],
                ident[0 : HD + 1, 0 : HD + 1],
            )
            rcp = finp.tile([P, 1], F32, tag="rcp")
            nc.vector.reciprocal(out=rcp, in_=tr[:, HD : HD + 1])
            if h == 0:  # ACT has headroom mid-loop; DVE idles at the tail
                nc.scalar.activation(
                    out=attn[:, h, ds(g * HD, HD)], in_=tr[:, 0:HD],
                    func=AF.Identity, scale=rcp,
                )
            else:
                nc.vector.tensor_scalar_mul(
                    out=attn[:, h, ds(g * HD, HD)], in0=tr[:, 0:HD], scalar1=rcp
                )

        def h1_unit(h):
            # h1 = attn + x for head h, transposed into h1T/h1Tb columns
            nc.vector.tensor_add(
                out=h1[:, h, :], in0=attn[:, h, :], in1=xl[:, h, :]
            )
            for dc in range(8):
                tps = pst.tile([P, P], F32, tag="t")
                nc.tensor.transpose(tps, h1[:, h, ds(dc * P, P)], ident)
                nc.vector.tensor_copy(out=h1T[:, dc, ds(h * P, P)], in_=tps)
                if h == 0:
                    nc.scalar.activation(
                        out=h1Tb[:, dc, ds(h * P, P)], in_=tps, func=AF.Copy
                    )
                else:
                    nc.vector.tensor_copy(
                        out=h1Tb[:, dc, ds(h * P, P)], in_=tps
                    )

        # Q projection (g-pairs; dout 128 at a time), pre-scaled 1/8
        # host-side; the first 8 A-units interleave so DVE starts early.
        with (
            tc.tile_pool(name="wq", bufs=2) as wqp,
            tc.tile_pool(name="xtl", bufs=1) as xtlp,
            tc.psum_pool(name="ps_q", bufs=2) as psq,
        ):
            xtl = xtlp.tile([P, 8, RQ], F32R)
            nc.sync.dma_start(
                out=xtl, in_=io["xTloc"].rearrange("(j p) n -> p j n", p=P)
            )
            wqr = io["wqT"].rearrange("(j p) c -> p j c", p=P)
            for half in range(2):
                wq = wqp.tile([P, 8, 4 * P], F32R, tag="wq")
                nc.sync.dma_start(
                    out=wq, in_=wqr[:, :, ds(half * 4 * P, 4 * P)]
                )
                for t in range(4):
                    gp = half * 4 + t  # g-pair index; g = 2*gp, 2*gp+1
                    ps = psq.tile([P, RQ], F32, tag="pq")
                    for dx in range(8):
                        nc.tensor.matmul(
                            ps,
                            lhsT=wq[:, dx, ds(t * P, P)],
                            rhs=xtl[:, dx, :],
                            start=(dx == 0),
                            stop=(dx == 7),
                        )
                    for gh in range(2):
                        g = 2 * gp + gh
                        nc.scalar.activation(
                            out=qTnm[0:HD, :, g, :],
                            in_=ps[ds(gh * HD, HD), :].rearrange(
                                "p (h q) -> p h q", h=2
                            ),
                            func=AF.Identity,
                            bias=bqs[:, g : g + 1],
                        )
                    if half == 0:
                        a_mm(0, 2 * gp)
                        a_mm(0, 2 * gp + 1)

        # chunk c consumes g-set G(c); a_mm runs 2 chunks ahead (G0/G1 were
        # emitted in the Q loop), a_nt 1 ahead, fin one behind; h1 for a head
        # follows its last fin batch.
        chunks = [(h, qc) for h in range(2) for qc in range(4)]
        gsets = [[(h, qc * 4 + j) for j in range(4)] for h, qc in chunks]
        with (
            tc.psum_pool(name="ps_st", bufs=2) as psst_,
            tc.psum_pool(name="ps_n", bufs=1) as psn_,
        ):
            nonlocal_pools["psst"] = psst_
            nonlocal_pools["psn"] = psn_
            for hg in gsets[0]:
                a_nt(*hg)
            for c, (h, qc) in enumerate(chunks):
                if c == 1:
                    # FFN weight chunks 0-1 stream while DMA engines idle
                    wtiles.append(_ffn_weight_dma(tc, io, wsp, 0))
                    wtiles.append(_ffn_weight_dma(tc, io, wsp, 1))
                if c + 2 < len(chunks):
                    for hg in gsets[c + 2]:
                        a_mm(*hg)
                if c + 1 < len(chunks):
                    for hg in gsets[c + 1]:
                        a_nt(*hg)
                b_chunk(h, qc)
                if c > 0:
                    for hg in gsets[c - 1]:
                        fin_unit(*hg)
                    if gsets[c - 1][-1][1] == 15:  # last g of a head done
                        h1_unit(gsets[c - 1][0][0])
            for hg in gsets[-1]:
                fin_unit(*hg)
            h1_unit(1)


def _ffn_weight_dma(tc, io, wsp, chunk):
    # one batched DMA per 1024-col bf16 weight chunk; chunks 0-3 = W1
    # quarters, 4-7 = W2 quarters
    nc = tc.nc
    w = wsp.tile([P, 8, 8 * P], BF16, tag="wbig")
    if chunk < 4:
        src = io["w1T"].rearrange("(j p) c -> p j c", p=P)[
            :, :, ds(chunk * 8 * P, 8 * P)
        ]
    else:
        q2 = chunk - 4
        src = io["w2T"][ds(q2 * 8 * P, 8 * P), :].rearrange(
            "(j p) c -> p j c", p=P
        )
    nc.sync.dma_start(out=w, in_=src)
    return w


def _ffn_phase(tc, io, b1s, b2s, h1T, h1Tb, wsp, wtiles):
    nc = tc.nc
    with (
        tc.tile_pool(name="ffn", bufs=1) as fp,
        tc.tile_pool(name="ffn_sm", bufs=3) as fsm,
    ):
        hid = fp.tile([P, 32, RQ], BF16)
        with tc.psum_pool(name="ps_f", bufs=3) as psf:
            for q4 in range(4):  # W1 column quarters [128, 8, 1024]
                wtiles.append(_ffn_weight_dma(tc, io, wsp, q4 + 2))
                w1q = wtiles[q4]
                for f in range(8):
                    ff = q4 * 8 + f
                    ps = psf.tile([P, RQ], F32, tag="fps")
                    for dc in range(8):
                        nc.tensor.matmul(
                            ps,
                            lhsT=w1q[:, dc, ds(f * P, P)],
                            rhs=h1Tb[:, dc, :],
                            start=(dc == 0),
                            stop=(dc == 7),
                        )
                    nc.scalar.activation(
                        out=hid[:, ff, :], in_=ps, func=AF.Relu,
                        bias=b1s[:, ff : ff + 1],
                    )
        with tc.psum_pool(name="ps_y", bufs=1) as psy:
            yaccs = [
                psy.tile([P, RQ], F32, tag=f"y{dy}", name=f"yacc{dy}")
                for dy in range(8)
            ]
            for q2 in range(4):  # W2 ffc-quarters [128, 8, 1024]
                if q2 + 6 < 8:
                    wtiles.append(_ffn_weight_dma(tc, io, wsp, q2 + 6))
                w2q = wtiles[4 + q2]
                for dy in range(8):
                    for fc in range(8):
                        nc.tensor.matmul(
                            yaccs[dy],
                            lhsT=w2q[:, fc, ds(dy * P, P)],
                            rhs=hid[:, q2 * 8 + fc, :],
                            start=(q2 == 0 and fc == 0),
                            stop=(q2 == 3 and fc == 7),
                        )
            for dy in range(8):
                ysb = fsm.tile([P, RQ], F32, tag="ysb")
                nc.scalar.activation(
                    out=ysb, in_=yaccs[dy], func=AF.Identity,
                    bias=b2s[:, dy : dy + 1],
                )
                osb = fsm.tile([P, RQ], F32, tag="osb")
                nc.vector.tensor_add(out=osb, in0=ysb, in1=h1T[:, dy, :])
                nc.sync.dma_start(out=io["outT"][ds(dy * P, P), :], in_=osb)


def _build():
    nc = bacc.Bacc(
        "TRN2", target_bir_lowering=False, debug=False, num_devices=NCORES
    )
    io = {}
    def inp(name, shape, dt=F32):
        io[name] = nc.dram_tensor(name, shape, dt, kind="ExternalInput").ap()
    inp("xT", [D, N], BF16)
    inp("xTloc", [D, RQ], F32R)
    inp("xloc", [RQ, D])
    inp("wqT", [D, D], F32R)
    inp("wkT", [D, P], BF16)
    inp("wvT", [D, P], BF16)
    inp("w1T", [D, FF], BF16)
    inp("w2T", [FF, D], BF16)
    inp("bq2d", [HD, 16])
    inp("bk2d", [HD, 2])
    inp("bv2d", [HD, 2])
    inp("b12d", [P, 32])
    inp("b22d", [P, 8])
    io["outT"] = nc.dram_tensor("outT", [D, RQ], F32, kind="ExternalOutput").ap()
    with tile.TileContext(nc) as tc:
        _tile_kernel(tc, io)
    nc.compile()
    return nc


_CACHE = {}


def _get_nc():
    if "nc" not in _CACHE:
        _CACHE["nc"] = _build()
    return _CACHE["nc"]


_BF16NP = mybir.dt.np(BF16)


def _rows_for_core(c):
    # local row r = h*128 + Q  ->  global n = Q*16 + 2c + h
    r = np.arange(RQ)
    h, Q = r // P, r % P
    return Q * 16 + 2 * c + h


def make_in_maps(inputs):
    x = np.ascontiguousarray(np.asarray(inputs["x"], np.float32)[0])
    xT = np.ascontiguousarray(x.T)

    wqT8 = np.ascontiguousarray((np.asarray(inputs["Wq"], np.float32) / 8.0).T)
    bq8 = np.asarray(inputs["bq"], np.float32) / 8.0
    wkT = np.asarray(inputs["Wk"], np.float32).T  # [din, dout]
    wvT = np.asarray(inputs["Wv"], np.float32).T
    bk = np.asarray(inputs["bk"], np.float32)
    bv = np.asarray(inputs["bv"], np.float32)

    def b2d(b, k):
        return np.ascontiguousarray(np.asarray(b, np.float32).reshape(k, P).T)

    common = {
        "xT": xT.astype(_BF16NP),
        "wqT": wqT8,
        "bq2d": np.ascontiguousarray(bq8.reshape(16, HD).T),  # [f, g]
        "w1T": np.ascontiguousarray(
            np.asarray(inputs["W1"], np.float32).T.astype(_BF16NP)
        ),
        "w2T": np.ascontiguousarray(
            np.asarray(inputs["W2"], np.float32).T.astype(_BF16NP)
        ),
        "b12d": b2d(inputs["b1"], 32),
        "b22d": b2d(inputs["b2"], 8),
    }
    in_maps = []
    for c in range(NCORES):
        rows = _rows_for_core(c)
        sl = slice(c * P, (c + 1) * P)
        m = dict(common)
        m["xTloc"] = np.ascontiguousarray(xT[:, rows])
        m["xloc"] = np.ascontiguousarray(x[rows])
        m["wkT"] = np.ascontiguousarray(wkT[:, sl].astype(_BF16NP))
        m["wvT"] = np.ascontiguousarray(wvT[:, sl].astype(_BF16NP))
        m["bk2d"] = np.ascontiguousarray(bk[sl].reshape(2, HD).T)
        m["bv2d"] = np.ascontiguousarray(bv[sl].reshape(2, HD).T)
        in_maps.append(m)
    return in_maps


def kernel(**inputs):
    nc = _get_nc()
    res = run_bass_kernel_spmd(nc, make_in_maps(inputs), core_ids=list(range(NCORES)))
    out = np.empty((1, N, D), np.float32)
    for c in range(NCORES):
        out[0, _rows_for_core(c), :] = res.results[c]["outT"].T
    return out


# revision 54
# speedup vs baseline: 1.0174x; 1.0049x over previous
"""Blockwise-parallel transformer layer on 8 TRN2 NeuronCores.

Sharding: by kv-head (the reference's einsum ties kv-head to seq pos mod 16).
Core c owns heads {2c, 2c+1} and the 256 seq rows n with n%16 in {2c, 2c+1}.
K/V projections therefore only need the 128-wide Wk/Wv column slice for the
core's two heads (8x less replicated GEMM work than seq-sharding, and no
collectives).  Scores run twice: q-major for the per-block max (DVE
reduce_max), kc-major for exp/num so no e-transposes are needed; the block
max is broadcast into the kc-major psum with an indicator matmul, and den
falls out of a ones-column augmented into V.

Shapes (hardcoded): x (1, 2048, 1024); Wq/Wk/Wv (1024, 1024); W1 (4096,
1024); W2 (1024, 4096); H=16 heads * HD=64; KB=16 kv blocks of 128.
"""

import sys
from contextlib import ExitStack

import numpy as np

for _p in ("/opt/trn_rl_repo", "/root/.axon_site/_ro/trn_rl_repo"):
    if _p not in sys.path:
        sys.path.append(_p)

import concourse.bass as bass  # noqa: E402
import concourse.tile as tile  # noqa: E402
from concourse import bacc, mybir  # noqa: E402
from concourse._compat import with_exitstack  # noqa: E402
from concourse.bass import ds  # noqa: E402
from concourse.bass_utils import run_bass_kernel_spmd  # noqa: E402
from concourse.masks import make_identity  # noqa: E402

D = 1024
H = 16
HD = 64
FF = 4096
N = 2048
KB = 16
NCORES = 8
RQ = N // NCORES  # 256 local rows
P = 128

F32 = mybir.dt.float32
F32R = mybir.dt.float32r
BF16 = mybir.dt.bfloat16
AX = mybir.AxisListType
AF = mybir.ActivationFunctionType


@with_exitstack
def _tile_kernel(ctx: ExitStack, tc: tile.TileContext, io: dict):
    nc = tc.nc

    consts = ctx.enter_context(tc.tile_pool(name="consts", bufs=1))
    ident = consts.tile([P, P], F32)
    make_identity(nc, ident)
    identr = consts.tile([P, P], F32R)
    nc.scalar.activation(out=identr, in_=ident, func=AF.Copy)
    bqs = consts.tile([HD, 16], F32)
    bks = consts.tile([HD, 2], F32)
    bvs = consts.tile([HD, 2], F32)
    b1s = consts.tile([P, 32], F32)
    b2s = consts.tile([P, 8], F32)
    nc.sync.dma_start(out=bqs, in_=io["bq2d"])
    nc.sync.dma_start(out=bks, in_=io["bk2d"])
    nc.sync.dma_start(out=bvs, in_=io["bv2d"])
    nc.sync.dma_start(out=b1s, in_=io["b12d"])
    nc.sync.dma_start(out=b2s, in_=io["b22d"])

    persist = ctx.enter_context(tc.tile_pool(name="persist", bufs=1))
    attn = persist.tile([P, 2, D], F32)  # [Q, h, (g,f)]
    xl = persist.tile([P, 2, D], F32)  # local x rows (residual), [Q, h, d]
    h1 = persist.tile([P, 2, D], F32)
    h1T = persist.tile([P, 8, RQ], F32)
    h1Tb = persist.tile([P, 8, RQ], BF16)  # bf16 copy for GEMM1 rhs
    nc.sync.dma_start(out=xl[:, 0, :], in_=io["xloc"][0:P, :])
    nc.sync.dma_start(out=xl[:, 1, :], in_=io["xloc"][P : 2 * P, :])

    with tc.tile_pool(name="kvq", bufs=1) as kvp:
        # kTind rows 0-63: k for head h; rows 64-79: block indicator.
        # qTnm rows 0-63: q channels; rows 64-79: -blockmax rows (pass A).
        kTind = kvp.tile([HD + KB, 2, N], F32R)
        vaug = kvp.tile([P, KB, 2, HD + 1], F32R)
        qTnm = kvp.tile([HD + KB, 2, 16, P], F32R)
        nm = kvp.tile([P, 2, 16, KB], F32)  # -max per [Q, h, g, K]
        # IND[j, kc] = 1 iff j == kc//128 — block-max broadcast rows 64-79
        with tc.tile_pool(name="tmp_ind", bufs=1) as tmpp:
            indf = tmpp.tile([KB, KB * P], F32)
            nc.gpsimd.memset(indf, 1.0)
            nc.gpsimd.affine_select(
                out=indf, in_=indf, compare_op=mybir.AluOpType.is_ge, fill=0.0,
                base=0, pattern=[[1, KB * P]], channel_multiplier=-P,
            )
            nc.gpsimd.affine_select(
                out=indf, in_=indf, compare_op=mybir.AluOpType.is_ge, fill=0.0,
                base=P - 1, pattern=[[-1, KB * P]], channel_multiplier=P,
            )
            for h in range(2):
                nc.scalar.activation(
                    out=kTind[HD : HD + KB, h, :], in_=indf, func=AF.Copy
                )
        with tc.tile_pool(name="wstream", bufs=2) as wsp:
            h1p = (ident, xl, h1, h1T, h1Tb)
            wtiles = []
            _attention(
                tc, io, kvp, identr, bqs, bks, bvs, kTind, vaug, qTnm, nm,
                attn, wsp, h1p, wtiles,
            )
            _ffn_phase(tc, io, b1s, b2s, h1T, h1Tb, wsp, wtiles)


def _kvq_proj(tc, io, kvp, identr, bks, bvs, kT2, vaug):
    # kT2 is the full kTind tile; only rows 0-63 are written here.
    nc = tc.nc
    NCH = 4
    CW = N // NCH  # 512
    ones32 = kvp.tile([P, KB * 2], F32)
    nc.gpsimd.memset(ones32, 1.0)
    # den ones column at f=64 of every (K, h) slot
    nc.scalar.activation(
        out=vaug[:, :, :, HD : HD + 1].rearrange("p a b c -> p (a b c)"),
        in_=ones32,
        func=AF.Copy,
    )

    # K/V projections: only this core's 2-head dout slice (128 cols)
    with (
        tc.tile_pool(name="vt2", bufs=1) as vt2p,
        tc.tile_pool(name="wkv", bufs=1) as wp,
        tc.tile_pool(name="xs", bufs=2) as xsp,
        tc.psum_pool(name="ps_kv", bufs=2) as pskv,
    ):
        vT2 = vt2p.tile([HD, 2, N], F32R)
        wk = wp.tile([P, 8, P], BF16)
        wv = wp.tile([P, 8, P], BF16)
        nc.sync.dma_start(out=wk, in_=io["wkT"].rearrange("(j p) c -> p j c", p=P))
        nc.sync.dma_start(out=wv, in_=io["wvT"].rearrange("(j p) c -> p j c", p=P))
        xTr = io["xT"].rearrange("(j p) n -> p j n", p=P)
        for c in range(NCH):
            xc = xsp.tile([P, 8, CW], BF16, tag="xc")
            nc.sync.dma_start(out=xc, in_=xTr[:, :, ds(c * CW, CW)])
            for w, bias_t, dst in ((wk, bks, kT2), (wv, bvs, vT2)):
                ps = pskv.tile([P, CW], F32, tag="pkv")
                for dx in range(8):
                    nc.tensor.matmul(
                        ps,
                        lhsT=w[:, dx, :],
                        rhs=xc[:, dx, :],
                        start=(dx == 0),
                        stop=(dx == 7),
                    )
                for h in range(2):
                    nc.vector.tensor_scalar_add(
                        out=dst[0:HD, h, ds(c * CW, CW)],
                        in0=ps[ds(h * HD, HD), :],
                        scalar1=bias_t[:, h : h + 1],
                    )

        # vT2 -> vaug (n-major) via PE transposes
        with tc.psum_pool(name="ps_vt", bufs=2) as psvt:
            for h in range(2):
                for K in range(KB):
                    vt = psvt.tile([P, HD], F32R, tag="vt")
                    nc.tensor.transpose(
                        vt, vT2[:, h, ds(K * P, P)], identr[0:HD, 0:HD]
                    )
                    nc.vector.tensor_copy(out=vaug[:, K, h, 0:HD], in_=vt)

def _attention(
    tc, io, kvp, identr, bqs, bks, bvs, kTind, vaug, qTnm, nm, attn, wsp,
    h1p, wtiles,
):
    """Pass A (q-major scores -> per-block -max, DVE-bound) software-pipelined
    under pass B (kc-major scores+max-broadcast in ONE matmul via the
    indicator rows stacked at partitions 64-79, then exp -> num/den matmul).

    Emission order interleaves A-units (one g at a time) ahead of the B
    chunks that consume their -max rows, so DVE reductions hide under PE.
    The Q projection is fused in front, with the first 8 A-units interleaved
    so DVE starts early; h1/h1T for each head are built as soon as its
    attention output finalizes.
    """
    nc = tc.nc
    ident, xl, h1, h1T, h1Tb = h1p
    nsb = kvp.tile([HD + 1, 2, N], F32)  # [f(+den), h, (g,Q)]

    _kvq_proj(tc, io, kvp, identr, bks, bvs, kTind, vaug)

    with (
        tc.tile_pool(name="et", bufs=3) as etp,
        tc.tile_pool(name="fin", bufs=3) as finp,
        tc.psum_pool(name="ps_a", bufs=2) as psa,
        tc.psum_pool(name="ps_t", bufs=1) as pst,
    ):

        def a_mm(h, g):
            # q-major scores for (h, g); -max per kv block (DVE)
            for sh in range(4):
                s = psa.tile([P, 4 * P], F32, tag="s")
                nc.tensor.matmul(
                    s,
                    lhsT=qTnm[0:HD, h, g, :],
                    rhs=kTind[0:HD, h, ds(sh * 512, 512)],
                    start=True,
                    stop=True,
                )
                nc.vector.reduce_max(
                    out=nm[:, h, g, ds(sh * 4, 4)],
                    in_=s.rearrange("p (b f) -> p b f", f=P),
                    axis=AX.X,
                    negate=True,
                )

        def a_nt(h, g):
            # -max rows into qTnm partitions 64-79 (deferred so PE does not
            # stall in-queue behind the DVE reduces)
            nt = pst.tile([P, P], F32, tag="t")
            nc.tensor.transpose(nt[0:KB, :], nm[:, h, g, :], ident)
            nc.scalar.activation(
                out=qTnm[HD : HD + KB, h, g, :], in_=nt[0:KB, :], func=AF.Copy
            )

        nonlocal_pools = {}

        def b_chunk(h, qc):
            # 512 q-things (4 g), all kv blocks in pairs; one exp per pair;
            # st pairs emitted ahead of num so PE never waits on ACT.
            psst = nonlocal_pools["psst"]
            psn = nonlocal_pools["psn"]
            nacc = psn.tile([HD + 1, 512], F32, tag="nacc")
            ets = []

            def mm_st(p):
                st = psst.tile([P, 2, 512], F32, tag="st")
                for i in range(2):
                    nc.tensor.matmul(
                        st[:, i, :],
                        lhsT=kTind[:, h, ds((2 * p + i) * P, P)],
                        rhs=qTnm[:, h, ds(qc * 4, 4), :],
                        start=True,
                        stop=True,
                    )
                et = etp.tile([P, 2, 512], F32R, tag="et")
                nc.scalar.activation(out=et, in_=st, func=AF.Exp)
                ets.append(et)

            def mm_num(p):
                for i in range(2):
                    K = 2 * p + i
                    nc.tensor.matmul(
                        nacc,
                        lhsT=vaug[:, K, h, :],
                        rhs=ets[p][:, i, :],
                        start=(K == 0),
                        stop=(K == KB - 1),
                    )

            NP2 = KB // 2
            mm_st(0)
            mm_st(1)
            for p in range(NP2):
                if p + 2 < NP2:
                    mm_st(p + 2)
                mm_num(p)
            nc.scalar.activation(
                out=nsb[:, h, ds(qc * 512, 512)], in_=nacc, func=AF.Identity
            )

        def fin_unit(h, g):
            # transpose num/den back to Q-partitions, scale by 1/den
            tr = pst.tile([P, P], F32, tag="t")
            nc.tensor.transpose(
                tr[:, 0 : HD + 1], nsb[:, h, ds(g * P, P)],
                ident[0 : HD + 1, 0 : HD + 1],
            )
            rcp = finp.tile([P, 1], F32, tag="rcp")
            nc.vector.reciprocal(out=rcp, in_=tr[:, HD : HD + 1])
            if h == 0:  # ACT has headroom mid-loop; DVE idles at the tail
                nc.scalar.activation(
                    out=attn[:, h, ds(g * HD, HD)], in_=tr[:, 0:HD],
                    func=AF.Identity, scale=rcp,
                )
            else:
                nc.vector.tensor_scalar_mul(
                    out=attn[:, h, ds(g * HD, HD)], in0=tr[:, 0:HD], scalar1=rcp
                )

        def h1_seg(h, qc):
            # h1 = attn + x for the 256-col segment just finalized, and its
            # two h1T/h1Tb column blocks — spreads the residual+transpose
            # work through the loop instead of a serial tail
            sl = ds(qc * 2 * P, 2 * P)
            nc.vector.tensor_add(
                out=h1[:, h, sl], in0=attn[:, h, sl], in1=xl[:, h, sl]
            )
            for dc in (qc * 2, qc * 2 + 1):
                tps = pst.tile([P, P], F32, tag="t")
                nc.tensor.transpose(tps, h1[:, h, ds(dc * P, P)], ident)
                nc.vector.tensor_copy(out=h1T[:, dc, ds(h * P, P)], in_=tps)
                if h == 0:
                    nc.scalar.activation(
                        out=h1Tb[:, dc, ds(h * P, P)], in_=tps, func=AF.Copy
                    )
                else:
                    nc.vector.tensor_copy(
                        out=h1Tb[:, dc, ds(h * P, P)], in_=tps
                    )

        # Q projection (g-pairs; dout 128 at a time), pre-scaled 1/8
        # host-side; the first 8 A-units interleave so DVE starts early.
        with (
            tc.tile_pool(name="wq", bufs=2) as wqp,
            tc.tile_pool(name="xtl", bufs=1) as xtlp,
            tc.psum_pool(name="ps_q", bufs=2) as psq,
        ):
            xtl = xtlp.tile([P, 8, RQ], BF16)
            nc.sync.dma_start(
                out=xtl, in_=io["xTloc"].rearrange("(j p) n -> p j n", p=P)
            )
            wqr = io["wqT"].rearrange("(j p) c -> p j c", p=P)
            for half in range(2):
                wq = wqp.tile([P, 8, 4 * P], BF16, tag="wq")
                nc.sync.dma_start(
                    out=wq, in_=wqr[:, :, ds(half * 4 * P, 4 * P)]
                )
                for t in range(4):
                    gp = half * 4 + t  # g-pair index; g = 2*gp, 2*gp+1
                    ps = psq.tile([P, RQ], F32, tag="pq")
                    for dx in range(8):
                        nc.tensor.matmul(
                            ps,
                            lhsT=wq[:, dx, ds(t * P, P)],
                            rhs=xtl[:, dx, :],
                            start=(dx == 0),
                            stop=(dx == 7),
                        )
                    for gh in range(2):
                        g = 2 * gp + gh
                        nc.scalar.activation(
                            out=qTnm[0:HD, :, g, :],
                            in_=ps[ds(gh * HD, HD), :].rearrange(
                                "p (h q) -> p h q", h=2
                            ),
                            func=AF.Identity,
                            bias=bqs[:, g : g + 1],
                        )
                    if half == 0:
                        a_mm(0, 2 * gp)
                        a_mm(0, 2 * gp + 1)

        # chunk c consumes g-set G(c); a_mm runs 2 chunks ahead (G0/G1 were
        # emitted in the Q loop), a_nt 1 ahead, fin one behind; h1 for a head
        # follows its last fin batch.
        chunks = [(h, qc) for h in range(2) for qc in range(4)]
        gsets = [[(h, qc * 4 + j) for j in range(4)] for h, qc in chunks]
        with (
            tc.psum_pool(name="ps_st", bufs=2) as psst_,
            tc.psum_pool(name="ps_n", bufs=1) as psn_,
        ):
            nonlocal_pools["psst"] = psst_
            nonlocal_pools["psn"] = psn_
            for hg in gsets[0]:
                a_nt(*hg)
            for c, (h, qc) in enumerate(chunks):
                if c == 1:
                    # FFN weight chunks 0-1 stream while DMA engines idle
                    wtiles.append(_ffn_weight_dma(tc, io, wsp, 0))
                    wtiles.append(_ffn_weight_dma(tc, io, wsp, 1))
                if c + 2 < len(chunks):
                    for hg in gsets[c + 2]:
                        a_mm(*hg)
                if c + 1 < len(chunks):
                    for hg in gsets[c + 1]:
                        a_nt(*hg)
                b_chunk(h, qc)
                if c > 0:
                    ph, pqc = chunks[c - 1]
                    for hg in gsets[c - 1]:
                        fin_unit(*hg)
                    h1_seg(ph, pqc)
            ph, pqc = chunks[-1]
            for hg in gsets[-1]:
                fin_unit(*hg)
            h1_seg(ph, pqc)


def _ffn_weight_dma(tc, io, wsp, chunk):
    # one batched DMA per 1024-col bf16 weight chunk; chunks 0-3 = W1
    # quarters, 4-7 = W2 quarters
    nc = tc.nc
    w = wsp.tile([P, 8, 8 * P], BF16, tag="wbig")
    if chunk < 4:
        src = io["w1T"].rearrange("(j p) c -> p j c", p=P)[
            :, :, ds(chunk * 8 * P, 8 * P)
        ]
    else:
        q2 = chunk - 4
        src = io["w2T"][ds(q2 * 8 * P, 8 * P), :].rearrange(
            "(j p) c -> p j c", p=P
        )
    nc.sync.dma_start(out=w, in_=src)
    return w


def _ffn_phase(tc, io, b1s, b2s, h1T, h1Tb, wsp, wtiles):
    nc = tc.nc
    with (
        tc.tile_pool(name="ffn", bufs=1) as fp,
        tc.tile_pool(name="ffn_sm", bufs=3) as fsm,
    ):
        hid = fp.tile([P, 32, RQ], BF16)
        with tc.psum_pool(name="ps_f", bufs=3) as psf:
            for q4 in range(4):  # W1 column quarters [128, 8, 1024]
                wtiles.append(_ffn_weight_dma(tc, io, wsp, q4 + 2))
                w1q = wtiles[q4]
                for f in range(8):
                    ff = q4 * 8 + f
                    ps = psf.tile([P, RQ], F32, tag="fps")
                    for dc in range(8):
                        nc.tensor.matmul(
                            ps,
                            lhsT=w1q[:, dc, ds(f * P, P)],
                            rhs=h1Tb[:, dc, :],
                            start=(dc == 0),
                            stop=(dc == 7),
                        )
                    nc.scalar.activation(
                        out=hid[:, ff, :], in_=ps, func=AF.Relu,
                        bias=b1s[:, ff : ff + 1],
                    )
        with tc.psum_pool(name="ps_y", bufs=1) as psy:
            yaccs = [
                psy.tile([P, RQ], F32, tag=f"y{dy}", name=f"yacc{dy}")
                for dy in range(8)
            ]
            for q2 in range(4):  # W2 ffc-quarters [128, 8, 1024]
                if q2 + 6 < 8:
                    wtiles.append(_ffn_weight_dma(tc, io, wsp, q2 + 6))
                w2q = wtiles[4 + q2]
                for dy in range(8):
                    for fc in range(8):
                        nc.tensor.matmul(
                            yaccs[dy],
                            lhsT=w2q[:, fc, ds(dy * P, P)],
                            rhs=hid[:, q2 * 8 + fc, :],
                            start=(q2 == 0 and fc == 0),
                            stop=(q2 == 3 and fc == 7),
                        )
            for dy in range(8):
                ysb = fsm.tile([P, RQ], F32, tag="ysb")
                nc.scalar.activation(
                    out=ysb, in_=yaccs[dy], func=AF.Identity,
                    bias=b2s[:, dy : dy + 1],
                )
                osb = fsm.tile([P, RQ], F32, tag="osb")
                nc.vector.tensor_add(out=osb, in0=ysb, in1=h1T[:, dy, :])
                nc.sync.dma_start(out=io["outT"][ds(dy * P, P), :], in_=osb)


def _build():
    nc = bacc.Bacc(
        "TRN2", target_bir_lowering=False, debug=False, num_devices=NCORES
    )
    io = {}
    def inp(name, shape, dt=F32):
        io[name] = nc.dram_tensor(name, shape, dt, kind="ExternalInput").ap()
    inp("xT", [D, N], BF16)
    inp("xTloc", [D, RQ], BF16)
    inp("xloc", [RQ, D])
    inp("wqT", [D, D], BF16)
    inp("wkT", [D, P], BF16)
    inp("wvT", [D, P], BF16)
    inp("w1T", [D, FF], BF16)
    inp("w2T", [FF, D], BF16)
    inp("bq2d", [HD, 16])
    inp("bk2d", [HD, 2])
    inp("bv2d", [HD, 2])
    inp("b12d", [P, 32])
    inp("b22d", [P, 8])
    io["outT"] = nc.dram_tensor("outT", [D, RQ], F32, kind="ExternalOutput").ap()
    with tile.TileContext(nc) as tc:
        _tile_kernel(tc, io)
    nc.compile()
    return nc


_CACHE = {}


def _get_nc():
    if "nc" not in _CACHE:
        _CACHE["nc"] = _build()
    return _CACHE["nc"]


_BF16NP = mybir.dt.np(BF16)


def _rows_for_core(c):
    # local row r = h*128 + Q  ->  global n = Q*16 + 2c + h
    r = np.arange(RQ)
    h, Q = r // P, r % P
    return Q * 16 + 2 * c + h


def make_in_maps(inputs):
    x = np.ascontiguousarray(np.asarray(inputs["x"], np.float32)[0])
    xT = np.ascontiguousarray(x.T)

    wqT8 = np.ascontiguousarray((np.asarray(inputs["Wq"], np.float32) / 8.0).T)
    bq8 = np.asarray(inputs["bq"], np.float32) / 8.0
    wkT = np.asarray(inputs["Wk"], np.float32).T  # [din, dout]
    wvT = np.asarray(inputs["Wv"], np.float32).T
    bk = np.asarray(inputs["bk"], np.float32)
    bv = np.asarray(inputs["bv"], np.float32)

    def b2d(b, k):
        return np.ascontiguousarray(np.asarray(b, np.float32).reshape(k, P).T)

    common = {
        "xT": xT.astype(_BF16NP),
        "wqT": wqT8.astype(_BF16NP),
        "bq2d": np.ascontiguousarray(bq8.reshape(16, HD).T),  # [f, g]
        "w1T": np.ascontiguousarray(
            np.asarray(inputs["W1"], np.float32).T.astype(_BF16NP)
        ),
        "w2T": np.ascontiguousarray(
            np.asarray(inputs["W2"], np.float32).T.astype(_BF16NP)
        ),
        "b12d": b2d(inputs["b1"], 32),
        "b22d": b2d(inputs["b2"], 8),
    }
    in_maps = []
    for c in range(NCORES):
        rows = _rows_for_core(c)
        sl = slice(c * P, (c + 1) * P)
        m = dict(common)
        m["xTloc"] = np.ascontiguousarray(xT[:, rows].astype(_BF16NP))
        m["xloc"] = np.ascontiguousarray(x[rows])
        m["wkT"] = np.ascontiguousarray(wkT[:, sl].astype(_BF16NP))
        m["wvT"] = np.ascontiguousarray(wvT[:, sl].astype(_BF16NP))
        m["bk2d"] = np.ascontiguousarray(bk[sl].reshape(2, HD).T)
        m["bv2d"] = np.ascontiguousarray(bv[sl].reshape(2, HD).T)
        in_maps.append(m)
    return in_maps


def kernel(**inputs):
    nc = _get_nc()
    res = run_bass_kernel_spmd(nc, make_in_maps(inputs), core_ids=list(range(NCORES)))
    out = np.empty((1, N, D), np.float32)
    for c in range(NCORES):
        out[0, _rows_for_core(c), :] = res.results[c]["outT"].T
    return out
